# revision 1
# baseline (speedup 1.0000x reference)
"""Bradley-Terry loss kernel for Trainium2 — Chebyshev/PE design.

loss = sum_{i!=j} W[i,j] * softplus(b_j - b_i)
     = sum_{m,l} A[m,l] * z[m,l] - ln2 * trace(W),
  z[m,l] = sum_ij W_ij T_m(x_i) T_l(x_j),  x = (b - c)/h in [-1,1]

softplus(h*(y-x)) is approximated by a degree-63 tensor-product Chebyshev
expansion (max abs error ~1e-13 on the beta range), so the whole O(N^2)
contraction is a matmul: per core, TensorE computes
  Y[m, j] = sum_{i in shard} W[i, j] * T_m(x_i)
with the Chebyshev basis C as the stationary operand.  W streams in bf16
(rounding washes out: verified ~3e-7 end-to-end); the basis is kept at
double-bf16 precision by stacking hi/lo columns [C_hi | C_lo] -> M=128.
PSUM accumulates in fp32 over the 8 row-tiles.  The per-core Y [128, 8192]
is gathered and the tiny O(N*d) remainder (hi+lo combine, stage-2 with the
exact fp64 basis, A-contraction) runs in float64 on the host.
DMA is the critical path: 32MB of W in + 4MB of Y out per core.
"""

import numpy as np
import ml_dtypes

import concourse.bacc as bacc
import concourse.bass as bass
import concourse.mybir as mybir
from concourse import tile
from concourse.bass_utils import run_bass_kernel_spmd

N = 8192
NCORES = 8
R = N // NCORES            # 1024 rows per core
P = 128                    # SBUF partitions
TROWS = R // P             # 8 row-tiles per core
CHALF = 2048               # column group processed per PSUM generation
NHALF = N // CHALF
SLAB = 512                 # PSUM bank free size (fp32)
NSLAB = CHALF // SLAB      # 4 tags x 2 bufs -> 8 PSUM banks
DEG = 63
M1 = DEG + 1               # 64 chebyshev coefficients
_NEG_LN2 = -float(np.log(2.0))

_cached_nc = None


def _cheb_vals(x, deg):
    out = np.empty((len(x), deg + 1), dtype=np.float64)
    out[:, 0] = 1.0
    if deg >= 1:
        out[:, 1] = x
    for k in range(2, deg + 1):
        out[:, k] = 2 * x * out[:, k - 1] - out[:, k - 2]
    return out


def _cheb2d_coeffs(f, deg):
    n = deg + 1
    theta = (np.arange(n) + 0.5) * np.pi / n
    pts = np.cos(theta)
    F = f(pts[:, None], pts[None, :])
    Tm = np.cos(np.outer(np.arange(n), theta))
    A = (2.0 / n) * Tm @ F @ ((2.0 / n) * Tm).T
    A[0, :] /= 2
    A[:, 0] /= 2
    return A


def _build():
    nc = bacc.Bacc(
        "TRN2",
        target_bir_lowering=False,
        debug=False,
        enable_asserts=False,
        num_devices=NCORES,
    )
    f32 = mybir.dt.float32
    bf16 = mybir.dt.bfloat16
    w = nc.dram_tensor("w", [R, N], f32, kind="ExternalInput")
    crows = nc.dram_tensor("crows", [P, TROWS * P], bf16, kind="ExternalInput")
    diag = nc.dram_tensor("diag", [R], f32, kind="ExternalInput")
    y = nc.dram_tensor("y", [P, N], f32, kind="ExternalOutput")
    dsum = nc.dram_tensor("dsum", [P, 1], f32, kind="ExternalOutput")

    with tile.TileContext(nc) as tc:
        with (
            tc.tile_pool(name="consts", bufs=1) as consts,
            tc.tile_pool(name="wpool", bufs=4) as wpool,
            tc.tile_pool(name="wbpool", bufs=4) as wbpool,
            tc.tile_pool(name="ypool", bufs=2) as ypool,
            tc.tile_pool(name="psum", bufs=2, space="PSUM") as pspool,
            tc.tile_pool(name="small", bufs=2) as small,
        ):
            crows_sb = consts.tile([P, TROWS * P], bf16)
            nc.gpsimd.dma_start(crows_sb[:], crows.ap())
            diag_sb = consts.tile([P, TROWS], f32)
            nc.gpsimd.dma_start(diag_sb[:], diag.ap().rearrange("(t p) -> p t", p=P))

            for ch in range(NHALF):
                ps = [
                    pspool.tile([P, SLAB], f32, tag=f"ps{s}", name=f"ps{s}_{ch}")
                    for s in range(NSLAB)
                ]
                for t in range(TROWS):
                    wt = wpool.tile([P, CHALF], f32, tag="w")
                    nc.sync.dma_start(
                        wt[:],
                        w.ap()[t * P : (t + 1) * P, ch * CHALF : (ch + 1) * CHALF],
                    )
                    wb = wbpool.tile([P, CHALF], bf16, tag="wb")
                    nc.vector.tensor_copy(wb[:], wt[:])
                    lhsT = crows_sb[:, t * P : (t + 1) * P]
                    for s in range(NSLAB):
                        nc.tensor.matmul(
                            ps[s][:],
                            lhsT,
                            wb[:, s * SLAB : (s + 1) * SLAB],
                            start=(t == 0),
                            stop=(t == TROWS - 1),
                        )
                yh = ypool.tile([P, CHALF], f32, tag="y")
                for s in range(NSLAB):
                    # ScalarE is idle here and sits closer to PSUM
                    nc.scalar.copy(yh[:, s * SLAB : (s + 1) * SLAB], ps[s][:])
                # y writes go out on the Activation HWDGE queue so the sync
                # queue stays a pure W-read stream (no read/write turnaround)
                nc.scalar.dma_start(y.ap()[:, ch * CHALF : (ch + 1) * CHALF], yh[:])

            # dsum[p] = -ln2 * sum_t diag[p, t]
            dscr = small.tile([P, TROWS], f32, tag="dscr")
            dacc = small.tile([P, 1], f32, tag="dacc")
            nc.vector.scalar_tensor_tensor(
                out=dscr[:],
                in0=diag_sb[:],
                scalar=_NEG_LN2,
                in1=diag_sb[:],
                op0=mybir.AluOpType.mult,
                op1=mybir.AluOpType.bypass,
                accum_out=dacc[:],
            )
            nc.sync.dma_start(dsum.ap(), dacc[:])

    nc.compile()
    return nc


def _get_nc():
    global _cached_nc
    if _cached_nc is None:
        _cached_nc = _build()
    return _cached_nc


def kernel(win_matrix, betas, _trace=False):
    win_matrix = np.asarray(win_matrix, dtype=np.float32)
    betas = np.asarray(betas, dtype=np.float32)
    nc = _get_nc()

    b64 = betas.astype(np.float64)
    lo, hi = float(b64.min()), float(b64.max())
    c = 0.5 * (lo + hi)
    h = max(0.5 * (hi - lo) * 1.000001, 1e-12)
    x = (b64 - c) / h
    A = _cheb2d_coeffs(lambda X, Y: np.logaddexp(0.0, h * (Y - X)), DEG)
    C = _cheb_vals(x, DEG)                       # [N, 64] f64
    C_hi = C.astype(ml_dtypes.bfloat16)
    C_lo = (C - C_hi.astype(np.float64)).astype(ml_dtypes.bfloat16)

    dvals = np.ascontiguousarray(np.diagonal(win_matrix))
    in_maps = []
    for cc in range(NCORES):
        rows = slice(cc * R, (cc + 1) * R)
        stacked = np.concatenate(
            [C_hi[rows].reshape(TROWS, P, M1), C_lo[rows].reshape(TROWS, P, M1)],
            axis=2,
        )  # [t, p, 128]
        crows_np = np.ascontiguousarray(
            stacked.transpose(1, 0, 2).reshape(P, TROWS * P)
        )
        in_maps.append(
            {
                "w": np.ascontiguousarray(win_matrix[rows]),
                "crows": crows_np,
                "diag": np.ascontiguousarray(dvals[rows]),
            }
        )
    res = run_bass_kernel_spmd(
        nc, in_maps, core_ids=list(range(NCORES)), trace=_trace
    )

    Ysum = np.zeros((M1, N), dtype=np.float64)
    dtot = 0.0
    for cc in range(NCORES):
        yv = res.results[cc]["y"].astype(np.float64)
        Ysum += yv[:M1] + yv[M1:]
        dtot += float(res.results[cc]["dsum"].astype(np.float64).sum())
    z = Ysum @ C                                  # [64, 64]
    total = float((A * z).sum()) + dtot
    if _trace:
        kernel.last_results = res
    return np.array(total, dtype=np.float32)



# revision 2
# speedup vs baseline: 2.5854x; 2.5854x over previous
"""Bradley-Terry loss kernel for Trainium2 — fp8 DoubleRow Chebyshev design.

loss = sum_{i!=j} W[i,j] * softplus(b_j - b_i)
     = sum_{m,l} A[m,l] * z[m,l] - ln2 * trace(W),
  z[m,l] = sum_ij W_ij T_m(x_i) T_l(x_j),  x = (b - c)/h in [-1,1]

softplus(h*(y-x)) is approximated by a degree-31 tensor-product Chebyshev
expansion (approx error ~6e-8 end-to-end).  Per core, TensorE computes
  Y[m, j] = sum_{i in shard} W[i, j] * T_m(x_i)
with the Chebyshev basis as the stationary operand in fp8(e4m3) DoubleRow
mode (two contraction rows per cycle, 256 W-rows per matmul group).  The
basis is kept at double-fp8 precision by stacking hi/lo columns
[C_hi | C_lo] -> M=64, which fits DoubleRow's 64-partition output limit.
W itself streams as fp8(e4m3): quantization error on U[0,1] entries is
zero-mean and washes out over the 67M-term sum (measured ~1e-4 rel end
to end, vs the 2e-2 gate).

The j-contraction with D[m,j] = sum_l A[m,l] T_l(x_j) (computed on host
in f64, shipped bf16) runs on-device: VectorE multiplies each PSUM slab
by D and row-reduces, so only a tiny [64, 16] accumulator leaves the
chip instead of the 4MB Y.  Total HBM traffic per core ~9.1MB (vs 36MB
for the f32 baseline): W 8MB in + D 1MB in + basis/diag/acc noise.
"""

import numpy as np
import ml_dtypes

import concourse.bacc as bacc
import concourse.bass as bass
import concourse.mybir as mybir
from concourse import tile
from concourse.bass_utils import run_bass_kernel_spmd

N = 8192
NCORES = 8
R = N // NCORES            # 1024 rows per core
P = 128                    # SBUF partitions
T2 = R // (2 * P)          # 4 double-row tiles of 256 rows per core
CHALF = 2048               # column group processed per PSUM generation
NHALF = N // CHALF
SLAB = 512                 # PSUM bank free size (fp32)
NSLAB = CHALF // SLAB      # 4 tags x 2 bufs -> 8 PSUM banks
DEG = 31
M1 = DEG + 1               # 32 chebyshev coefficients
MSTACK = 2 * M1            # hi|lo stacked -> 64 matmul output partitions
_NEG_LN2 = -float(np.log(2.0))

_cached_nc = None


def _cheb_vals(x, deg):
    out = np.empty((len(x), deg + 1), dtype=np.float64)
    out[:, 0] = 1.0
    if deg >= 1:
        out[:, 1] = x
    for k in range(2, deg + 1):
        out[:, k] = 2 * x * out[:, k - 1] - out[:, k - 2]
    return out


def _cheb2d_coeffs(f, deg):
    n = deg + 1
    theta = (np.arange(n) + 0.5) * np.pi / n
    pts = np.cos(theta)
    F = f(pts[:, None], pts[None, :])
    Tm = np.cos(np.outer(np.arange(n), theta))
    A = (2.0 / n) * Tm @ F @ ((2.0 / n) * Tm).T
    A[0, :] /= 2
    A[:, 0] /= 2
    return A


def _build():
    nc = bacc.Bacc(
        "TRN2",
        target_bir_lowering=False,
        debug=False,
        enable_asserts=False,
        num_devices=NCORES,
    )
    f32 = mybir.dt.float32
    bf16 = mybir.dt.bfloat16
    fp8 = mybir.dt.float8e4
    w = nc.dram_tensor("w", [R, N], fp8, kind="ExternalInput")
    crows = nc.dram_tensor("crows", [P, T2, 2, MSTACK], fp8, kind="ExternalInput")
    dmat = nc.dram_tensor("dmat", [MSTACK, N], bf16, kind="ExternalInput")
    diag = nc.dram_tensor("diag", [R], f32, kind="ExternalInput")
    acc_out = nc.dram_tensor("acc", [MSTACK, NHALF * NSLAB], f32, kind="ExternalOutput")
    dsum = nc.dram_tensor("dsum", [P, 1], f32, kind="ExternalOutput")

    with tile.TileContext(nc) as tc:
        with (
            tc.tile_pool(name="consts", bufs=1) as consts,
            tc.tile_pool(name="wpool", bufs=4) as wpool,
            tc.tile_pool(name="psum", bufs=2, space="PSUM") as pspool,
            tc.tile_pool(name="scr", bufs=2) as scrpool,
            tc.tile_pool(name="small", bufs=2) as small,
        ):
            crows_sb = consts.tile([P, T2, 2, MSTACK], fp8)
            nc.gpsimd.dma_start(crows_sb[:], crows.ap())
            diag_sb = consts.tile([P, R // P], f32)
            nc.gpsimd.dma_start(diag_sb[:], diag.ap().rearrange("(t p) -> p t", p=P))
            # D ships on the Activation HWDGE queue so the sync queue stays
            # a pure W-read stream
            dmat_sb = consts.tile([MSTACK, N], bf16)
            nc.scalar.dma_start(dmat_sb[:], dmat.ap())
            acc = consts.tile([MSTACK, NHALF * NSLAB], f32)

            for ch in range(NHALF):
                ps = [
                    pspool.tile([MSTACK, SLAB], f32, tag=f"ps{s}", name=f"ps{s}_{ch}")
                    for s in range(NSLAB)
                ]
                for t2 in range(T2):
                    wt = wpool.tile([P, 2, CHALF], fp8, tag="w")
                    nc.sync.dma_start(
                        wt[:],
                        w.ap()[
                            t2 * 2 * P : (t2 + 1) * 2 * P,
                            ch * CHALF : (ch + 1) * CHALF,
                        ].rearrange("(r p) j -> p r j", p=P),
                    )
                    lhsT = crows_sb[:, t2, :, :]
                    for s in range(NSLAB):
                        nc.tensor.matmul(
                            ps[s][:],
                            lhsT,
                            wt[:, :, s * SLAB : (s + 1) * SLAB],
                            start=(t2 == 0),
                            stop=(t2 == T2 - 1),
                            perf_mode=mybir.MatmulPerfMode.DoubleRow,
                        )
                for s in range(NSLAB):
                    scr = scrpool.tile([MSTACK, SLAB], f32, tag="scr")
                    col = ch * NSLAB + s
                    nc.vector.scalar_tensor_tensor(
                        out=scr[:],
                        in0=ps[s][:],
                        scalar=1.0,
                        in1=dmat_sb[:, ch * CHALF + s * SLAB : ch * CHALF + (s + 1) * SLAB],
                        op0=mybir.AluOpType.mult,
                        op1=mybir.AluOpType.mult,
                        accum_out=acc[:, col : col + 1],
                    )

            # dsum[p] = -ln2 * sum_t diag[p, t]
            dscr = small.tile([P, R // P], f32, tag="dscr")
            dacc = small.tile([P, 1], f32, tag="dacc")
            nc.vector.scalar_tensor_tensor(
                out=dscr[:],
                in0=diag_sb[:],
                scalar=_NEG_LN2,
                in1=diag_sb[:],
                op0=mybir.AluOpType.mult,
                op1=mybir.AluOpType.bypass,
                accum_out=dacc[:],
            )
            nc.sync.dma_start(dsum.ap(), dacc[:])
            nc.sync.dma_start(acc_out.ap(), acc[:])

    nc.compile()
    return nc


def _get_nc():
    global _cached_nc
    if _cached_nc is None:
        _cached_nc = _build()
    return _cached_nc


def kernel(win_matrix, betas, _trace=False):
    win_matrix = np.asarray(win_matrix, dtype=np.float32)
    betas = np.asarray(betas, dtype=np.float32)
    nc = _get_nc()

    b64 = betas.astype(np.float64)
    lo, hi = float(b64.min()), float(b64.max())
    c = 0.5 * (lo + hi)
    h = max(0.5 * (hi - lo) * 1.000001, 1e-12)
    x = (b64 - c) / h
    A = _cheb2d_coeffs(lambda X, Y: np.logaddexp(0.0, h * (Y - X)), DEG)
    C = _cheb_vals(x, DEG)                       # [N, 32] f64
    fp8 = ml_dtypes.float8_e4m3
    C_hi = C.astype(fp8)
    C_lo = (C - C_hi.astype(np.float64)).astype(fp8)
    C_st = np.concatenate([C_hi, C_lo], axis=1)  # [N, 64] fp8
    D31 = A @ C.T                                # [32, N] f64
    Dp = np.ascontiguousarray(
        np.concatenate([D31, D31], axis=0).astype(ml_dtypes.bfloat16)
    )                                            # [64, N] bf16

    W8 = win_matrix.astype(fp8)                  # [N, N] fp8
    dvals = np.ascontiguousarray(np.diagonal(win_matrix))
    in_maps = []
    for cc in range(NCORES):
        rows = slice(cc * R, (cc + 1) * R)
        # crows[p, t2, r, m] = C_st[cc*R + t2*256 + r*128 + p, m]
        crows_np = np.ascontiguousarray(
            C_st[rows].reshape(T2, 2, P, MSTACK).transpose(2, 0, 1, 3)
        )
        in_maps.append(
            {
                "w": W8[rows],
                "crows": crows_np,
                "dmat": Dp,
                "diag": np.ascontiguousarray(dvals[rows]),
            }
        )
    res = run_bass_kernel_spmd(
        nc, in_maps, core_ids=list(range(NCORES)), trace=_trace
    )

    total = 0.0
    for cc in range(NCORES):
        total += float(res.results[cc]["acc"].astype(np.float64).sum())
        total += float(res.results[cc]["dsum"].astype(np.float64).sum())
    if _trace:
        kernel.last_results = res
    return np.array(total, dtype=np.float32)
